# revision 42
# baseline (speedup 1.0000x reference)
"""Sparse-attention Trainium2 kernel (nn_Attention_81398220193933), v2.

Strategy (tensor-parallel over heads, 2 heads per NeuronCore), fp16:
  - Logits are computed pre-scaled for a Schraudolph fp16 exp: the QK
    matmul produces p[k,q] = C0*(s+b_k) where C0 = 2^10/ln2, s = q.k/8,
    and b_k is the per-key bias (softmax bias; -200 for masked keys).
    The bias rides in two augmented contract rows (hi/lo split of
    b_k*C0/256 against constant 256 rows in Q^T), so masking/bias cost
    nothing on-device and stay fp16-exact.
  - exp is split across TWO engines per tile [128k, 1024q]:
      ScalarE: ACTIVATE Exp, scale=1/C0 -> exact exp in fp16 (~1.1us)
      DVE:     tensor_scalar (p + C1C) max 0 -> int16, bit-cast fp16
               = Schraudolph exp (~1.2us, +-3% sawtooth, mostly
               cancelled by softmax normalization).
    ~3/8 of tiles go to the DVE, keeping ScalarE under the PE pace
    (alone it would be the bottleneck at 82us).
  - AV is V-stationary/W-moving: acc^T[65, q] += V_tile^T @ W^T with
    col 64 of the stationary = ones -> row 64 accumulates the softmax
    denominator. 2 matmuls of N=512 per (tile, head); PSUM exactly
    fits 4 acc banks + 2x2 score banks.
  - Output ships unnormalized [65, S] f32 per (batch, head) via a DVE
    PSUM->SBUF copy + DMA; the host does the final divide + transpose
    (correctness is checked on the assembled full output).
"""

import numpy as np

import concourse.bass as bass
import concourse.mybir as mybir
import concourse.tile as tile
from concourse import bacc
from concourse.bass_utils import run_bass_kernel_spmd

B = 8
S = 1024
UNITS = 1024
H = 16
DH = 64
N_CORES = 8
KT = S // 128

F16 = mybir.dt.float16
F32 = mybir.dt.float32
I16 = mybir.dt.int16

C0 = float(2**10 / np.log(2))     # fp16 Schraudolph exponent scale
C1C = 15360.0 - 0.3           # 15*2^10 minus sawtooth-centering tweak
JUNK_N = 6                    # startup keep-warm matmul pairs


def _build_nc(kbs):
    nc = bacc.Bacc("TRN2", target_bir_lowering=False, debug=False,
                   num_devices=N_CORES)
    # qk[b]: 4 planes (qA, kA, qB, kB) of [66, S]; vt[b]: per-partition
    # front-packed kb*130 fp16 (stationary V tiles + ones column).
    qk = nc.dram_tensor("qk", [B, 4, 66, S], F16, kind="ExternalInput").ap()
    vt = nc.dram_tensor("vt", [B, 128, KT * 130], F16,
                        kind="ExternalInput").ap()
    o = nc.dram_tensor("o", [B, 2, 65, S], F32, kind="ExternalOutput").ap()

    with tile.TileContext(nc) as tc:
        with (
            tc.tile_pool(name="qk", bufs=2) as qk_pool,
            tc.tile_pool(name="v", bufs=2) as v_pool,
            tc.tile_pool(name="w", bufs=8) as w_pool,
            tc.tile_pool(name="st", bufs=4) as st_pool,
            tc.tile_pool(name="sc", bufs=2, space="PSUM") as sc_pool,
            tc.tile_pool(name="acc", bufs=2, space="PSUM") as acc_pool,
        ):
            # Preload the exp table-set while the first DMAs fly.
            wexp = qk_pool.tile([1, 8], F32, tag="wexp", name="wexp", bufs=1)
            nc.vector.memset(wexp[:], 0.0)
            nc.scalar.activation(wexp[:], wexp[:],
                                 mybir.ActivationFunctionType.Exp)

            # First batch small (warms the PE clock on real work at low
            # cost), then largest-first, smallest last (short tail).
            srt = sorted(range(B), key=lambda i: -kbs[i])
            order = [srt[-2]] + srt[:-2] + [srt[-1]]
            qkts, vts = {}, {}
            for bi, b in enumerate(order):
                qkt = qk_pool.tile([66, 4, S], F16, tag=f"qk{b}",
                                   name=f"qk{b}", bufs=1)
                if bi == 0:
                    # split so the first QK can start after half the load
                    nc.sync.dma_start(
                        out=qkt[:, 0:2, :],
                        in_=qk[b, 0:2].rearrange("f p s -> p f s"))
                    nc.sync.dma_start(
                        out=qkt[:, 2:4, :],
                        in_=qk[b, 2:4].rearrange("f p s -> p f s"))
                else:
                    nc.sync.dma_start(out=qkt[:],
                                      in_=qk[b].rearrange("f p s -> p f s"))
                qkts[b] = qkt
                vts[b] = v_pool.tile([128, kbs[b], 130], F16, tag=f"vt{b}",
                                     name=f"vt{b}", bufs=1)
                nc.sync.dma_start(
                    out=vts[b][:],
                    in_=vt[b, :, :kbs[b] * 130].rearrange(
                        "p (t c) -> p t c", c=130))

            recs = {}

            def emit_av(b, t, wts):
                rec = recs[b]
                kb = kbs[b]
                for h in range(2):
                    if t == 0:
                        rec["acc"][h] = acc_pool.tile(
                            [65, S], F32, tag="acc", name=f"ac{b}_{h}")
                    for c in range(2):
                        nc.tensor.matmul(
                            rec["acc"][h][:, 512 * c:512 * (c + 1)],
                            lhsT=vts[b][:, t, 65 * h:65 * h + 65],
                            rhs=wts[h][:, 512 * c:512 * (c + 1)],
                            start=(t == 0), stop=(t == kb - 1),
                        )
                    if t == kb - 1:
                        # epilogue per head, emitted right after that head's
                        # last AV matmul so its acc banks free one copy-time
                        # earlier (the next batch's AV reuses them).
                        # Copies stay on DVE: ScalarE must remain pure-Exp
                        # (ACTIVATE-Copy churns the activation table set, a
                        # 16KB DMA per reload that can gate the kernel end).
                        # Outputs alternate between the GpSimd and Sync
                        # hardware DMA queues so they drain in parallel; the
                        # final batch row-splits each DMA across both queues
                        # to shorten the terminal drain.
                        st = st_pool.tile([65, S], F32, tag="st", name="st")
                        nc.vector.tensor_copy(st[:], rec["acc"][h][:])
                        if b == order[-1]:
                            nc.gpsimd.dma_start(out=o[b, h, 0:33],
                                                in_=st[0:33, :])
                            nc.sync.dma_start(out=o[b, h, 33:65],
                                              in_=st[33:65, :])
                        else:
                            eng = nc.gpsimd if h == 0 else nc.sync
                            eng.dma_start(out=o[b, h], in_=st[:])

            # Startup junk matmuls: raise PE utilization while the first
            # DMAs land so the HAM clock ramps before real work begins.
            zj = qk_pool.tile([128, 512], F16, tag="zj", name="zj", bufs=1)
            nc.gpsimd.memset(zj[:], 0.0)
            for _ in range(JUNK_N):
                jt = sc_pool.tile([128, S], F32, tag="sc", name="jk")
                for c in range(2):
                    nc.tensor.matmul(
                        jt[:, 512 * c:512 * (c + 1)],
                        lhsT=zj[:, 0:128], rhs=zj[:],
                        start=True, stop=True, skip_group_check=True)

            steps = [(b, t) for b in order for t in range(kbs[b])]
            pending = []   # AV lags QK by AV_LAG steps: exp always has
            AV_LAG = 2     # a spare step to finish -> no PE bubble
            for si, (b, t) in enumerate(steps):
                if t == 0:
                    recs[b] = {"acc": [None, None]}
                wts = [None, None]
                # DVE-Schraudolph takes head 1 on 3 of 4 steps; the last
                # tile of each batch stays all-ScalarE so the DVE is free
                # for the PSUM-freeing epilogue copies at the boundary —
                # except the final step, where no later batch waits and
                # parallel engines shorten the tail.
                dve_step = (si % 4 != 3 and t != kbs[b] - 1) or \
                    si == len(steps) - 1
                for h in (0, 1):
                    qkt = qkts[b]
                    sc = sc_pool.tile([128, S], F32, tag="sc", name="sc")
                    for c in range(2):
                        nc.tensor.matmul(
                            sc[:, 512 * c:512 * (c + 1)],
                            lhsT=qkt[:, 2 * h + 1, t * 128:(t + 1) * 128],
                            rhs=qkt[:, 2 * h, 512 * c:512 * (c + 1)],
                            start=True, stop=True,
                        )
                    wt = w_pool.tile([128, S], F16, tag="w",
                                     name=f"w{b}_{t}_{h}")
                    if h == 1 and dve_step:
                        nc.vector.tensor_scalar(
                            wt[:].bitcast(I16), sc[:], C1C, 0.0,
                            mybir.AluOpType.add, mybir.AluOpType.max)
                    else:
                        nc.scalar.activation(
                            wt[:], sc[:], mybir.ActivationFunctionType.Exp,
                            scale=1.0 / C0)
                    wts[h] = wt
                pending.append((b, t, wts))
                if len(pending) > AV_LAG:
                    emit_av(*pending.pop(0))
            while pending:
                emit_av(*pending.pop(0))
    nc.compile()
    return nc


_NC_CACHE = {}


def _get_nc(kbs):
    key = tuple(kbs)
    if key not in _NC_CACHE:
        _NC_CACHE[key] = _build_nc(key)
    return _NC_CACHE[key]


def kernel(memory, query, b, seq_len):
    memory = np.asarray(memory)
    query = np.asarray(query)
    bias = np.asarray(b, dtype=np.float64)
    seq_len = np.asarray(seq_len).reshape(-1).astype(np.int64)

    sl = seq_len.copy()
    kbs = [int(min(KT, max(1, -(-int(s) // 128)))) if s > 0 else KT for s in sl]

    pos = np.arange(S)[None, :]
    valid = (pos < sl[:, None]) | (sl[:, None] == 0)
    bm = np.where(valid, bias[None, :], -200.0)          # [B, S]
    z = bm * C0
    z1 = (z / 256).astype(np.float16)
    z2 = (z / 256 - z1.astype(np.float64)).astype(np.float16)

    qh = (query.astype(np.float64) * (C0 / 8.0)).reshape(B, S, H, DH)
    kh = memory[:, :, :UNITS].reshape(B, S, H, DH)
    vh = memory[:, :, UNITS:].reshape(B, S, H, DH)

    qT = np.zeros((H, B, 66, S), dtype=np.float16)
    kT = np.zeros((H, B, 66, S), dtype=np.float16)
    qT[:, :, :64] = qh.astype(np.float16).transpose(2, 0, 3, 1)
    qT[:, :, 64] = 256.0
    qT[:, :, 65] = 256.0
    kT[:, :, :64] = kh.astype(np.float16).transpose(2, 0, 3, 1)
    kT[:, :, 64] = z1[None]
    kT[:, :, 65] = z2[None]

    # [B, S, H, DH] -> [B, 128, KT, H, DH]
    vtiles = np.ascontiguousarray(
        vh.astype(np.float16).reshape(B, KT, 128, H, DH)
        .transpose(0, 2, 1, 3, 4))

    in_maps = []
    for c in range(N_CORES):
        hA, hB = 2 * c, 2 * c + 1
        vE = np.zeros((B, 128, KT, 130), dtype=np.float16)
        for b in range(B):
            kb = kbs[b]
            vE[b, :, :kb, 0:64] = vtiles[b, :, :kb, hA, :]
            vE[b, :, :kb, 64] = 1.0
            vE[b, :, :kb, 65:129] = vtiles[b, :, :kb, hB, :]
            vE[b, :, :kb, 129] = 1.0
        qkp = np.stack([qT[hA], kT[hA], qT[hB], kT[hB]], axis=1)
        in_maps.append({
            "qk": np.ascontiguousarray(qkp),           # [B, 4, 66, S]
            "vt": vE.reshape(B, 128, KT * 130),
        })

    nc = _get_nc(kbs)
    res = run_bass_kernel_spmd(nc, in_maps, core_ids=list(range(N_CORES)))

    out = np.empty((B, S, UNITS), dtype=np.float32)
    for c in range(N_CORES):
        oc = res.results[c]["o"]                         # [B, 2, 65, S] f32
        num = oc[:, :, :64, :]                           # [B, 2, 64, S]
        den = oc[:, :, 64:65, :]
        r = (num / den).transpose(0, 3, 1, 2)            # [B, S, 2, 64]
        out[:, :, 128 * c:128 * (c + 1)] = r.reshape(B, S, 128)
    return out


# revision 43
# speedup vs baseline: 1.0130x; 1.0130x over previous
"""Sparse-attention Trainium2 kernel (nn_Attention_81398220193933), v2.

Strategy (tensor-parallel over heads, 2 heads per NeuronCore), fp16:
  - Logits are computed pre-scaled for a Schraudolph fp16 exp: the QK
    matmul produces p[k,q] = C0*(s+b_k) where C0 = 2^10/ln2, s = q.k/8,
    and b_k is the per-key bias (softmax bias; -200 for masked keys).
    The bias rides in two augmented contract rows (hi/lo split of
    b_k*C0/256 against constant 256 rows in Q^T), so masking/bias cost
    nothing on-device and stay fp16-exact.
  - exp is split across TWO engines per tile [128k, 1024q]:
      ScalarE: ACTIVATE Exp, scale=1/C0 -> exact exp in fp16 (~1.1us)
      DVE:     tensor_scalar (p + C1C) max 0 -> int16, bit-cast fp16
               = Schraudolph exp (~1.2us, +-3% sawtooth, mostly
               cancelled by softmax normalization).
    ~3/8 of tiles go to the DVE, keeping ScalarE under the PE pace
    (alone it would be the bottleneck at 82us).
  - AV is V-stationary/W-moving: acc^T[65, q] += V_tile^T @ W^T with
    col 64 of the stationary = ones -> row 64 accumulates the softmax
    denominator. 2 matmuls of N=512 per (tile, head); PSUM exactly
    fits 4 acc banks + 2x2 score banks.
  - Output ships unnormalized [65, S] f32 per (batch, head) via a DVE
    PSUM->SBUF copy + DMA; the host does the final divide + transpose
    (correctness is checked on the assembled full output).
"""

import numpy as np

import concourse.bass as bass
import concourse.mybir as mybir
import concourse.tile as tile
from concourse import bacc
from concourse.bass_utils import run_bass_kernel_spmd

B = 8
S = 1024
UNITS = 1024
H = 16
DH = 64
N_CORES = 8
KT = S // 128

F16 = mybir.dt.float16
F32 = mybir.dt.float32
I16 = mybir.dt.int16

C0 = float(2**10 / np.log(2))     # fp16 Schraudolph exponent scale
C1C = 15360.0 - 0.3           # 15*2^10 minus sawtooth-centering tweak
JUNK_N = 6                    # startup keep-warm matmul pairs


def _build_nc(kbs):
    nc = bacc.Bacc("TRN2", target_bir_lowering=False, debug=False,
                   num_devices=N_CORES)
    # qk[b]: 4 planes (qA, kA, qB, kB) of [66, S]; vt[b]: per-partition
    # front-packed kb*130 fp16 (stationary V tiles + ones column).
    qk = nc.dram_tensor("qk", [B, 4, 66, S], F16, kind="ExternalInput").ap()
    vt = nc.dram_tensor("vt", [B, 128, KT * 130], F16,
                        kind="ExternalInput").ap()
    o = nc.dram_tensor("o", [B, 2, 65, S], F32, kind="ExternalOutput").ap()

    with tile.TileContext(nc) as tc:
        with (
            tc.tile_pool(name="qk", bufs=2) as qk_pool,
            tc.tile_pool(name="v", bufs=2) as v_pool,
            tc.tile_pool(name="w", bufs=8) as w_pool,
            tc.tile_pool(name="st", bufs=4) as st_pool,
            tc.tile_pool(name="sc", bufs=2, space="PSUM") as sc_pool,
            tc.tile_pool(name="acc", bufs=2, space="PSUM") as acc_pool,
        ):
            # Preload the exp table-set while the first DMAs fly.
            wexp = qk_pool.tile([1, 8], F32, tag="wexp", name="wexp", bufs=1)
            nc.vector.memset(wexp[:], 0.0)
            nc.scalar.activation(wexp[:], wexp[:],
                                 mybir.ActivationFunctionType.Exp)

            # First batch small (warms the PE clock on real work at low
            # cost), then largest-first, smallest last (short tail).
            srt = sorted(range(B), key=lambda i: -kbs[i])
            order = [srt[-2]] + srt[:-2] + [srt[-1]]
            qkts, vts = {}, {}
            for bi, b in enumerate(order):
                qkt = qk_pool.tile([66, 4, S], F16, tag=f"qk{b}",
                                   name=f"qk{b}", bufs=1)
                if bi == 0:
                    # split so the first QK can start after half the load
                    nc.sync.dma_start(
                        out=qkt[:, 0:2, :],
                        in_=qk[b, 0:2].rearrange("f p s -> p f s"))
                    nc.sync.dma_start(
                        out=qkt[:, 2:4, :],
                        in_=qk[b, 2:4].rearrange("f p s -> p f s"))
                else:
                    nc.sync.dma_start(out=qkt[:],
                                      in_=qk[b].rearrange("f p s -> p f s"))
                qkts[b] = qkt
                vts[b] = v_pool.tile([128, kbs[b], 130], F16, tag=f"vt{b}",
                                     name=f"vt{b}", bufs=1)
                nc.sync.dma_start(
                    out=vts[b][:],
                    in_=vt[b, :, :kbs[b] * 130].rearrange(
                        "p (t c) -> p t c", c=130))

            recs = {}

            def emit_av(b, t, wts):
                rec = recs[b]
                kb = kbs[b]
                for h in range(2):
                    if t == 0:
                        rec["acc"][h] = acc_pool.tile(
                            [65, S], F32, tag="acc", name=f"ac{b}_{h}")
                    for c in range(2):
                        nc.tensor.matmul(
                            rec["acc"][h][:, 512 * c:512 * (c + 1)],
                            lhsT=vts[b][:, t, 65 * h:65 * h + 65],
                            rhs=wts[h][:, 512 * c:512 * (c + 1)],
                            start=(t == 0), stop=(t == kb - 1),
                        )
                    if t == kb - 1:
                        # epilogue per head, emitted right after that head's
                        # last AV matmul so its acc banks free one copy-time
                        # earlier (the next batch's AV reuses them).
                        # Copies stay on DVE: ScalarE must remain pure-Exp
                        # (ACTIVATE-Copy churns the activation table set, a
                        # 16KB DMA per reload that can gate the kernel end).
                        # Outputs alternate between the GpSimd and Sync
                        # hardware DMA queues so they drain in parallel; the
                        # final batch row-splits each DMA across both queues
                        # to shorten the terminal drain.
                        st = st_pool.tile([65, S], F32, tag="st", name="st")
                        nc.vector.tensor_copy(st[:], rec["acc"][h][:])
                        if b == order[-1]:
                            nc.gpsimd.dma_start(out=o[b, h, 0:33],
                                                in_=st[0:33, :])
                            nc.sync.dma_start(out=o[b, h, 33:65],
                                              in_=st[33:65, :])
                        else:
                            eng = nc.gpsimd if h == 0 else nc.sync
                            eng.dma_start(out=o[b, h], in_=st[:])

            # Startup junk matmuls: raise PE utilization while the first
            # DMAs land so the HAM clock ramps before real work begins.
            zj = qk_pool.tile([128, 512], F16, tag="zj", name="zj", bufs=1)
            nc.gpsimd.memset(zj[:], 0.0)
            for _ in range(JUNK_N):
                jt = sc_pool.tile([128, S], F32, tag="sc", name="jk")
                for c in range(2):
                    nc.tensor.matmul(
                        jt[:, 512 * c:512 * (c + 1)],
                        lhsT=zj[:, 0:128], rhs=zj[:],
                        start=True, stop=True, skip_group_check=True)

            steps = [(b, t) for b in order for t in range(kbs[b])]
            pending = []   # AV lags QK by AV_LAG steps: exp always has
            AV_LAG = 2     # a spare step to finish -> no PE bubble
            for si, (b, t) in enumerate(steps):
                if t == 0:
                    recs[b] = {"acc": [None, None]}
                wts = [None, None]
                # DVE-Schraudolph takes head 1 on 3 of 4 steps; the last
                # tile of each batch stays all-ScalarE so the DVE is free
                # for the PSUM-freeing epilogue copies at the boundary —
                dve_step = si % 4 != 3 and t != kbs[b] - 1
                for h in (0, 1):
                    qkt = qkts[b]
                    sc = sc_pool.tile([128, S], F32, tag="sc", name="sc")
                    for c in range(2):
                        nc.tensor.matmul(
                            sc[:, 512 * c:512 * (c + 1)],
                            lhsT=qkt[:, 2 * h + 1, t * 128:(t + 1) * 128],
                            rhs=qkt[:, 2 * h, 512 * c:512 * (c + 1)],
                            start=True, stop=True,
                        )
                    wt = w_pool.tile([128, S], F16, tag="w",
                                     name=f"w{b}_{t}_{h}")
                    if h == 1 and dve_step:
                        nc.vector.tensor_scalar(
                            wt[:].bitcast(I16), sc[:], C1C, 0.0,
                            mybir.AluOpType.add, mybir.AluOpType.max)
                    else:
                        nc.scalar.activation(
                            wt[:], sc[:], mybir.ActivationFunctionType.Exp,
                            scale=1.0 / C0)
                    wts[h] = wt
                pending.append((b, t, wts))
                if len(pending) > AV_LAG:
                    emit_av(*pending.pop(0))
            while pending:
                emit_av(*pending.pop(0))
    nc.compile()
    return nc


_NC_CACHE = {}


def _get_nc(kbs):
    key = tuple(kbs)
    if key not in _NC_CACHE:
        _NC_CACHE[key] = _build_nc(key)
    return _NC_CACHE[key]


def kernel(memory, query, b, seq_len):
    memory = np.asarray(memory)
    query = np.asarray(query)
    bias = np.asarray(b, dtype=np.float64)
    seq_len = np.asarray(seq_len).reshape(-1).astype(np.int64)

    sl = seq_len.copy()
    kbs = [int(min(KT, max(1, -(-int(s) // 128)))) if s > 0 else KT for s in sl]

    pos = np.arange(S)[None, :]
    valid = (pos < sl[:, None]) | (sl[:, None] == 0)
    bm = np.where(valid, bias[None, :], -200.0)          # [B, S]
    z = bm * C0
    z1 = (z / 256).astype(np.float16)
    z2 = (z / 256 - z1.astype(np.float64)).astype(np.float16)

    qh = (query.astype(np.float64) * (C0 / 8.0)).reshape(B, S, H, DH)
    kh = memory[:, :, :UNITS].reshape(B, S, H, DH)
    vh = memory[:, :, UNITS:].reshape(B, S, H, DH)

    qT = np.zeros((H, B, 66, S), dtype=np.float16)
    kT = np.zeros((H, B, 66, S), dtype=np.float16)
    qT[:, :, :64] = qh.astype(np.float16).transpose(2, 0, 3, 1)
    qT[:, :, 64] = 256.0
    qT[:, :, 65] = 256.0
    kT[:, :, :64] = kh.astype(np.float16).transpose(2, 0, 3, 1)
    kT[:, :, 64] = z1[None]
    kT[:, :, 65] = z2[None]

    # [B, S, H, DH] -> [B, 128, KT, H, DH]
    vtiles = np.ascontiguousarray(
        vh.astype(np.float16).reshape(B, KT, 128, H, DH)
        .transpose(0, 2, 1, 3, 4))

    in_maps = []
    for c in range(N_CORES):
        hA, hB = 2 * c, 2 * c + 1
        vE = np.zeros((B, 128, KT, 130), dtype=np.float16)
        for b in range(B):
            kb = kbs[b]
            vE[b, :, :kb, 0:64] = vtiles[b, :, :kb, hA, :]
            vE[b, :, :kb, 64] = 1.0
            vE[b, :, :kb, 65:129] = vtiles[b, :, :kb, hB, :]
            vE[b, :, :kb, 129] = 1.0
        qkp = np.stack([qT[hA], kT[hA], qT[hB], kT[hB]], axis=1)
        in_maps.append({
            "qk": np.ascontiguousarray(qkp),           # [B, 4, 66, S]
            "vt": vE.reshape(B, 128, KT * 130),
        })

    nc = _get_nc(kbs)
    res = run_bass_kernel_spmd(nc, in_maps, core_ids=list(range(N_CORES)))

    out = np.empty((B, S, UNITS), dtype=np.float32)
    for c in range(N_CORES):
        oc = res.results[c]["o"]                         # [B, 2, 65, S] f32
        num = oc[:, :, :64, :]                           # [B, 2, 64, S]
        den = oc[:, :, 64:65, :]
        r = (num / den).transpose(0, 3, 1, 2)            # [B, S, 2, 64]
        out[:, :, 128 * c:128 * (c + 1)] = r.reshape(B, S, 128)
    return out


# revision 44
# speedup vs baseline: 1.0162x; 1.0031x over previous
"""Sparse-attention Trainium2 kernel (nn_Attention_81398220193933), v2.

Strategy (tensor-parallel over heads, 2 heads per NeuronCore), fp16:
  - Logits are computed pre-scaled for a Schraudolph fp16 exp: the QK
    matmul produces p[k,q] = C0*(s+b_k) where C0 = 2^10/ln2, s = q.k/8,
    and b_k is the per-key bias (softmax bias; -200 for masked keys).
    The bias rides in two augmented contract rows (hi/lo split of
    b_k*C0/256 against constant 256 rows in Q^T), so masking/bias cost
    nothing on-device and stay fp16-exact.
  - exp is split across TWO engines per tile [128k, 1024q]:
      ScalarE: ACTIVATE Exp, scale=1/C0 -> exact exp in fp16 (~1.1us)
      DVE:     tensor_scalar (p + C1C) max 0 -> int16, bit-cast fp16
               = Schraudolph exp (~1.2us, +-3% sawtooth, mostly
               cancelled by softmax normalization).
    ~3/8 of tiles go to the DVE, keeping ScalarE under the PE pace
    (alone it would be the bottleneck at 82us).
  - AV is V-stationary/W-moving: acc^T[65, q] += V_tile^T @ W^T with
    col 64 of the stationary = ones -> row 64 accumulates the softmax
    denominator. 2 matmuls of N=512 per (tile, head); PSUM exactly
    fits 4 acc banks + 2x2 score banks.
  - Output ships unnormalized [65, S] f32 per (batch, head) via a DVE
    PSUM->SBUF copy + DMA; the host does the final divide + transpose
    (correctness is checked on the assembled full output).
"""

import numpy as np

import concourse.bass as bass
import concourse.mybir as mybir
import concourse.tile as tile
from concourse import bacc
from concourse.bass_utils import run_bass_kernel_spmd

B = 8
S = 1024
UNITS = 1024
H = 16
DH = 64
N_CORES = 8
KT = S // 128

F16 = mybir.dt.float16
F32 = mybir.dt.float32
I16 = mybir.dt.int16

C0 = float(2**10 / np.log(2))     # fp16 Schraudolph exponent scale
C1C = 15360.0 - 0.3           # 15*2^10 minus sawtooth-centering tweak
JUNK_N = 6                    # startup keep-warm matmul pairs


def _build_nc(kbs):
    nc = bacc.Bacc("TRN2", target_bir_lowering=False, debug=False,
                   num_devices=N_CORES)
    # qk[b]: 4 planes (qA, kA, qB, kB) of [66, S]; vt[b]: per-partition
    # front-packed kb*130 fp16 (stationary V tiles + ones column).
    qk = nc.dram_tensor("qk", [B, 4, 66, S], F16, kind="ExternalInput").ap()
    vt = nc.dram_tensor("vt", [B, 128, KT * 130], F16,
                        kind="ExternalInput").ap()
    o = nc.dram_tensor("o", [B, 2, 65, S], F32, kind="ExternalOutput").ap()

    with tile.TileContext(nc) as tc:
        with (
            tc.tile_pool(name="qk", bufs=2) as qk_pool,
            tc.tile_pool(name="v", bufs=2) as v_pool,
            tc.tile_pool(name="w", bufs=8) as w_pool,
            tc.tile_pool(name="st", bufs=4) as st_pool,
            tc.tile_pool(name="sc", bufs=2, space="PSUM") as sc_pool,
            tc.tile_pool(name="acc", bufs=2, space="PSUM") as acc_pool,
        ):
            # Preload the exp table-set while the first DMAs fly.
            wexp = qk_pool.tile([1, 8], F32, tag="wexp", name="wexp", bufs=1)
            nc.vector.memset(wexp[:], 0.0)
            nc.scalar.activation(wexp[:], wexp[:],
                                 mybir.ActivationFunctionType.Exp)

            # First batch small (warms the PE clock on real work at low
            # cost), then largest-first, smallest last (short tail).
            srt = sorted(range(B), key=lambda i: -kbs[i])
            order = [srt[-2]] + srt[:-2] + [srt[-1]]
            qkts, vts = {}, {}
            for bi, b in enumerate(order):
                qkt = qk_pool.tile([66, 4, S], F16, tag=f"qk{b}",
                                   name=f"qk{b}", bufs=1)
                if bi == 0:
                    # split so the first QK can start after half the load
                    nc.sync.dma_start(
                        out=qkt[:, 0:2, :],
                        in_=qk[b, 0:2].rearrange("f p s -> p f s"))
                    nc.sync.dma_start(
                        out=qkt[:, 2:4, :],
                        in_=qk[b, 2:4].rearrange("f p s -> p f s"))
                else:
                    nc.sync.dma_start(out=qkt[:],
                                      in_=qk[b].rearrange("f p s -> p f s"))
                qkts[b] = qkt
                vts[b] = v_pool.tile([128, kbs[b], 130], F16, tag=f"vt{b}",
                                     name=f"vt{b}", bufs=1)
                nc.sync.dma_start(
                    out=vts[b][:],
                    in_=vt[b, :, :kbs[b] * 130].rearrange(
                        "p (t c) -> p t c", c=130))

            recs = {}

            def emit_av(b, t, wts):
                rec = recs[b]
                kb = kbs[b]
                for h in range(2):
                    if t == 0:
                        rec["acc"][h] = acc_pool.tile(
                            [65, S], F32, tag="acc", name=f"ac{b}_{h}")
                    for c in range(2):
                        nc.tensor.matmul(
                            rec["acc"][h][:, 512 * c:512 * (c + 1)],
                            lhsT=vts[b][:, t, 65 * h:65 * h + 65],
                            rhs=wts[h][:, 512 * c:512 * (c + 1)],
                            start=(t == 0), stop=(t == kb - 1),
                        )
                    if t == kb - 1:
                        # epilogue per head, emitted right after that head's
                        # last AV matmul so its acc banks free one copy-time
                        # earlier (the next batch's AV reuses them).
                        # Copies stay on DVE: ScalarE must remain pure-Exp
                        # (ACTIVATE-Copy churns the activation table set, a
                        # 16KB DMA per reload that can gate the kernel end).
                        # Outputs alternate between the GpSimd and Sync
                        # hardware DMA queues so they drain in parallel; the
                        # final batch row-splits each DMA across both queues
                        # to shorten the terminal drain.
                        st = st_pool.tile([65, S], F32, tag="st", name="st")
                        nc.vector.tensor_copy(st[:], rec["acc"][h][:])
                        if b == order[-1]:
                            nc.gpsimd.dma_start(out=o[b, h, 0:33],
                                                in_=st[0:33, :])
                            nc.sync.dma_start(out=o[b, h, 33:65],
                                              in_=st[33:65, :])
                        else:
                            eng = nc.gpsimd if h == 0 else nc.sync
                            eng.dma_start(out=o[b, h], in_=st[:])

            # Startup junk matmuls: raise PE utilization while the first
            # DMAs land so the HAM clock ramps before real work begins.
            zj = qk_pool.tile([128, 512], F16, tag="zj", name="zj", bufs=1)
            nc.gpsimd.memset(zj[:], 0.0)
            for _ in range(JUNK_N):
                jt = sc_pool.tile([128, S], F32, tag="sc", name="jk")
                for c in range(2):
                    nc.tensor.matmul(
                        jt[:, 512 * c:512 * (c + 1)],
                        lhsT=zj[:, 0:128], rhs=zj[:],
                        start=True, stop=True, skip_group_check=True)

            steps = [(b, t) for b in order for t in range(kbs[b])]
            pending = []   # AV lags QK by AV_LAG steps: exp always has
            AV_LAG = 3     # a spare step to finish -> no PE bubble
            for si, (b, t) in enumerate(steps):
                if t == 0:
                    recs[b] = {"acc": [None, None]}
                wts = [None, None]
                # DVE-Schraudolph takes head 1 on 3 of 4 steps; the last
                # tile of each batch stays all-ScalarE so the DVE is free
                # for the PSUM-freeing epilogue copies at the boundary —
                dve_step = si % 4 != 3 and t != kbs[b] - 1
                for h in (0, 1):
                    qkt = qkts[b]
                    sc = sc_pool.tile([128, S], F32, tag="sc", name="sc")
                    for c in range(2):
                        nc.tensor.matmul(
                            sc[:, 512 * c:512 * (c + 1)],
                            lhsT=qkt[:, 2 * h + 1, t * 128:(t + 1) * 128],
                            rhs=qkt[:, 2 * h, 512 * c:512 * (c + 1)],
                            start=True, stop=True,
                        )
                    wt = w_pool.tile([128, S], F16, tag="w",
                                     name=f"w{b}_{t}_{h}")
                    if h == 1 and dve_step:
                        nc.vector.tensor_scalar(
                            wt[:].bitcast(I16), sc[:], C1C, 0.0,
                            mybir.AluOpType.add, mybir.AluOpType.max)
                    else:
                        nc.scalar.activation(
                            wt[:], sc[:], mybir.ActivationFunctionType.Exp,
                            scale=1.0 / C0)
                    wts[h] = wt
                pending.append((b, t, wts))
                if len(pending) > AV_LAG:
                    emit_av(*pending.pop(0))
            while pending:
                emit_av(*pending.pop(0))
    nc.compile()
    return nc


_NC_CACHE = {}


def _get_nc(kbs):
    key = tuple(kbs)
    if key not in _NC_CACHE:
        _NC_CACHE[key] = _build_nc(key)
    return _NC_CACHE[key]


def kernel(memory, query, b, seq_len):
    memory = np.asarray(memory)
    query = np.asarray(query)
    bias = np.asarray(b, dtype=np.float64)
    seq_len = np.asarray(seq_len).reshape(-1).astype(np.int64)

    sl = seq_len.copy()
    kbs = [int(min(KT, max(1, -(-int(s) // 128)))) if s > 0 else KT for s in sl]

    pos = np.arange(S)[None, :]
    valid = (pos < sl[:, None]) | (sl[:, None] == 0)
    bm = np.where(valid, bias[None, :], -200.0)          # [B, S]
    z = bm * C0
    z1 = (z / 256).astype(np.float16)
    z2 = (z / 256 - z1.astype(np.float64)).astype(np.float16)

    qh = (query.astype(np.float64) * (C0 / 8.0)).reshape(B, S, H, DH)
    kh = memory[:, :, :UNITS].reshape(B, S, H, DH)
    vh = memory[:, :, UNITS:].reshape(B, S, H, DH)

    qT = np.zeros((H, B, 66, S), dtype=np.float16)
    kT = np.zeros((H, B, 66, S), dtype=np.float16)
    qT[:, :, :64] = qh.astype(np.float16).transpose(2, 0, 3, 1)
    qT[:, :, 64] = 256.0
    qT[:, :, 65] = 256.0
    kT[:, :, :64] = kh.astype(np.float16).transpose(2, 0, 3, 1)
    kT[:, :, 64] = z1[None]
    kT[:, :, 65] = z2[None]

    # [B, S, H, DH] -> [B, 128, KT, H, DH]
    vtiles = np.ascontiguousarray(
        vh.astype(np.float16).reshape(B, KT, 128, H, DH)
        .transpose(0, 2, 1, 3, 4))

    in_maps = []
    for c in range(N_CORES):
        hA, hB = 2 * c, 2 * c + 1
        vE = np.zeros((B, 128, KT, 130), dtype=np.float16)
        for b in range(B):
            kb = kbs[b]
            vE[b, :, :kb, 0:64] = vtiles[b, :, :kb, hA, :]
            vE[b, :, :kb, 64] = 1.0
            vE[b, :, :kb, 65:129] = vtiles[b, :, :kb, hB, :]
            vE[b, :, :kb, 129] = 1.0
        qkp = np.stack([qT[hA], kT[hA], qT[hB], kT[hB]], axis=1)
        in_maps.append({
            "qk": np.ascontiguousarray(qkp),           # [B, 4, 66, S]
            "vt": vE.reshape(B, 128, KT * 130),
        })

    nc = _get_nc(kbs)
    res = run_bass_kernel_spmd(nc, in_maps, core_ids=list(range(N_CORES)))

    out = np.empty((B, S, UNITS), dtype=np.float32)
    for c in range(N_CORES):
        oc = res.results[c]["o"]                         # [B, 2, 65, S] f32
        num = oc[:, :, :64, :]                           # [B, 2, 64, S]
        den = oc[:, :, 64:65, :]
        r = (num / den).transpose(0, 3, 1, 2)            # [B, S, 2, 64]
        out[:, :, 128 * c:128 * (c + 1)] = r.reshape(B, S, 128)
    return out
